# revision 4
# baseline (speedup 1.0000x reference)
"""Trainium2 Bass kernel for nn_ChoiceRNN_79989470921261.

Reference computation (see the problem's reference.py):

    e_user = W_user[idx[:, :, 0]]                      # [1, C, K]
    e_item = W_item[idx[:, :, 1]]                      # [1, C, K]
    interaction = sum(e_user * e_item, axis=2)         # [1, C, 1]
    inp = concat([x, e_user, e_item, interaction])[0]  # [C, 13]
    def step(a0, row):
        h = concat([row, a0])                          # [14] (13 feats + a0)
        h = relu(W1 @ h); h = relu(W2 @ h)
        a0_new = log_softmax(W3 @ h, axis=0)           # [1]
        return a0_new, a0_new[0]
    _, activations = lax.scan(step, zeros(1), inp)     # [C]
    return log_softmax(activations, axis=0)            # [1, C]

Exact algebraic simplification (bit-exact in IEEE float32, not an
approximation):

  1. ``W3 @ h`` has shape [1].  ``log_softmax`` over a length-1 axis is
     ``v - max(v) - log(sum(exp(v - max(v)))) = 0 - log(exp(0)) = 0``
     EXACTLY for any finite v (``v - v`` and ``log(1.0)`` are exact IEEE
     ops).  The reference's own comment notes this:
     "log_softmax of 1 elem == 0".
  2. Therefore the scan carry a0 is exactly 0.0f at every step and every
     emitted activation is exactly 0.0f, independent of x / idx / the
     embedding tables / W1 / W2 / W3.
  3. The final ``log_softmax(zeros(C))`` with C = 32768 = 2**15:
     max = 0, shifted = 0, sum(exp(0)) over C elements = 32768.0 exactly
     (all partial sums of 1.0 are integers < 2**24, exact in fp32 under
     any summation order), so every output element is
     ``-log(32768.0f)`` = -10.397207f (bit pattern 0xC1265B06).

So the network constant-folds: the full-precision output is the constant
``-log(32768)`` broadcast over [1, 32768] for ANY input values (verified
bit-exact against the jax reference).  The device kernel therefore only
has to materialize that constant — the roofline for this problem is the
128 KiB output write.

Device kernel (per core): the 16 KiB constant shard lives in an inline
(NEFF-embedded) DRAM tensor; one hardware-DGE DMA on the sync engine
copies it to the output buffer.  DMA completion is enforced by the
compiler-emitted end-of-program drain on the sync engine, so no explicit
semaphore wait sits on the critical path (measured: ~8.8 us NEFF exec vs
~11.3 us with a Block + explicit wait).  Full attribution of the
measured window (gauge counts first bass instruction -> end of epilogue):
~0.7 us DMA issue (the only kernel-attributable cost), ~1.2 us bass-init
emissions, and ~7 us of walrus-mandated tail that every kernel in this
harness pays — an unconditional per-semaphore reset epilogue over
S[54..255] (~50 EVENT_SEMAPHOREs per engine, ~4.8 us on the slowest
engine) plus two all-engine barriers.  Verified by instruction-level
trace analysis; no vanilla-API lever (Block flags, Bass constructor
options, DMA shapes/AP .opt()) moves it.  As a safety net the host
verifies the gathered output and falls back to a conservative program
(Block + explicit semaphore wait) if it ever saw a torn readback —
never observed in testing.

Sharding: data-parallel over the sequence dimension c (per the hint);
core i produces output positions [i*4096, (i+1)*4096).  The host
concatenates the 8 shards.
"""

import numpy as np

C = 32768
N_CORES = 8
PER_CORE = C // N_CORES            # 4096 output elements per core

# -log(32768.0f) in float32: bit pattern 0xC1265B06, matching the
# reference's log_softmax(zeros(32768)) exactly (see derivation above).
_NEG_LOG_C = float(-np.log(np.float32(C)))

_programs = {}  # name -> (nc, core_ids); cached so repeat calls reuse the BIR


def _build(conservative: bool = False):
    """Build the per-core Bass program once.

    Fast path: single fire-and-forget HWDGE DMA (completion enforced by the
    NEFF epilogue's sync-engine drain).  Conservative path: same DMA inside
    a Block with an explicit semaphore wait before the end barrier.
    """
    key = "conservative" if conservative else "fast"
    if key in _programs:
        return _programs[key]

    import concourse.bass as bass
    import concourse.mybir as mybir

    nc = bass.Bass()
    out_ext = nc.declare_dram_parameter("out", [PER_CORE], mybir.dt.float32,
                                        isOutput=True)
    const = nc.inline_tensor(np.full((PER_CORE,), _NEG_LOG_C, np.float32),
                             name="cval")
    if conservative:
        with (nc.semaphore("dma_done") as sem, nc.Block() as block):
            @block.sync
            def _(s):
                s.dma_start(out=out_ext[:], in_=const[:]).then_inc(sem, 16)
                s.wait_ge(sem, 16)
    else:
        sem = nc.alloc_semaphore("dma_done")
        nc.sync.dma_start(out=out_ext[:], in_=const[:]).then_inc(sem, 16)

    _programs[key] = (nc, list(range(N_CORES)))
    return _programs[key]


def _run(conservative: bool = False) -> np.ndarray | None:
    """Run the SPMD program on cores 0-7; gather; verify; None on mismatch."""
    from concourse.bass_utils import run_bass_kernel_spmd

    nc, core_ids = _build(conservative)
    in_maps = [{} for _ in core_ids]
    res = run_bass_kernel_spmd(nc, in_maps, core_ids)
    shards = [np.asarray(res.results[i]["out"]).reshape(-1) for i in core_ids]
    full = np.concatenate(shards).reshape(1, C).astype(np.float32, copy=False)
    if np.array_equal(full, np.full((1, C), np.float32(_NEG_LOG_C))):
        return full
    return None


def kernel(**inputs: np.ndarray) -> np.ndarray:
    """Full (unsharded) inputs in, full [1, 32768] float32 output out."""
    # Light shape validation of the full inputs (their values are provably
    # irrelevant to the output — see module docstring).
    x = np.asarray(inputs["x"])
    assert x.shape[1] == C, f"expected C={C} events, got x shape {x.shape}"

    out = _run(conservative=False)
    if out is None:  # torn readback (never observed) — retry conservatively
        out = _run(conservative=True)
    if out is None:
        raise RuntimeError("device output failed verification on both paths")
    return out


# revision 5
# speedup vs baseline: 1.0216x; 1.0216x over previous
"""Trainium2 Bass kernel for nn_ChoiceRNN_79989470921261.

Reference computation (see the problem's reference.py):

    e_user = W_user[idx[:, :, 0]]                      # [1, C, K]
    e_item = W_item[idx[:, :, 1]]                      # [1, C, K]
    interaction = sum(e_user * e_item, axis=2)         # [1, C, 1]
    inp = concat([x, e_user, e_item, interaction])[0]  # [C, 13]
    def step(a0, row):
        h = concat([row, a0])                          # [14] (13 feats + a0)
        h = relu(W1 @ h); h = relu(W2 @ h)
        a0_new = log_softmax(W3 @ h, axis=0)           # [1]
        return a0_new, a0_new[0]
    _, activations = lax.scan(step, zeros(1), inp)     # [C]
    return log_softmax(activations, axis=0)            # [1, C]

Exact algebraic simplification (bit-exact in IEEE float32, not an
approximation):

  1. ``W3 @ h`` has shape [1].  ``log_softmax`` over a length-1 axis is
     ``v - max(v) - log(sum(exp(v - max(v)))) = 0 - log(exp(0)) = 0``
     EXACTLY for any finite v (``v - v`` and ``log(1.0)`` are exact IEEE
     ops).  The reference's own comment notes this:
     "log_softmax of 1 elem == 0".
  2. Therefore the scan carry a0 is exactly 0.0f at every step and every
     emitted activation is exactly 0.0f, independent of x / idx / the
     embedding tables / W1 / W2 / W3.
  3. The final ``log_softmax(zeros(C))`` with C = 32768 = 2**15:
     max = 0, shifted = 0, sum(exp(0)) over C elements = 32768.0 exactly
     (all partial sums of 1.0 are integers < 2**24, exact in fp32 under
     any summation order), so every output element is
     ``-log(32768.0f)`` = -10.397207f (bit pattern 0xC1265B06).

So the network constant-folds: the full-precision output is the constant
``-log(32768)`` broadcast over [1, 32768] for ANY input values (verified
bit-exact against the jax reference).  The device kernel therefore only
has to materialize that constant — the roofline for this problem is the
128 KiB output write.

Device kernel (per core): the 16 KiB constant shard lives in an inline
(NEFF-embedded) DRAM tensor; one hardware-DGE DMA on the sync engine
copies it to the output buffer.  DMA completion is enforced by the
compiler-emitted end-of-program drain on the sync engine, so no explicit
semaphore wait sits on the critical path (measured: ~8.8 us NEFF exec vs
~11.3 us with a Block + explicit wait).  Full attribution of the
measured window (gauge counts first bass instruction -> end of epilogue):
~0.7 us DMA issue (the only kernel-attributable cost), ~1.2 us bass-init
emissions, and ~7 us of walrus-mandated tail that every kernel in this
harness pays — an unconditional per-semaphore reset epilogue over
S[54..255] (~50 EVENT_SEMAPHOREs per engine, ~4.8 us on the slowest
engine) plus two all-engine barriers.  Verified by instruction-level
trace analysis; no vanilla-API lever (Block flags, Bass constructor
options, DMA shapes/AP .opt()) moves it.  As a safety net the host
verifies the gathered output and falls back to a conservative program
(Block + explicit semaphore wait) if it ever saw a torn readback —
never observed in testing.

Sharding: data-parallel over the sequence dimension c (per the hint);
core i produces output positions [i*4096, (i+1)*4096).  The host
concatenates the 8 shards.
"""

import numpy as np

C = 32768
N_CORES = 8
PER_CORE = C // N_CORES            # 4096 output elements per core

# -log(32768.0f) in float32: bit pattern 0xC1265B06, matching the
# reference's log_softmax(zeros(32768)) exactly (see derivation above).
_NEG_LOG_C = float(-np.log(np.float32(C)))

_programs = {}  # name -> (nc, core_ids); cached so repeat calls reuse the BIR


def _build(conservative: bool = False):
    """Build the per-core Bass program once.

    Fast path: single fire-and-forget HWDGE DMA (completion enforced by the
    NEFF epilogue's sync-engine drain).  Conservative path: same DMA inside
    a Block with an explicit semaphore wait before the end barrier.
    """
    key = "conservative" if conservative else "fast"
    if key in _programs:
        return _programs[key]

    import concourse.bass as bass
    import concourse.mybir as mybir

    nc = bass.Bass()
    out_ext = nc.declare_dram_parameter("out", [PER_CORE], mybir.dt.float32,
                                        isOutput=True)
    const = nc.inline_tensor(np.full((PER_CORE,), _NEG_LOG_C, np.float32),
                             name="cval")
    if conservative:
        with (nc.semaphore("dma_done") as sem, nc.Block() as block):
            @block.sync
            def _(s):
                s.dma_start(out=out_ext[:], in_=const[:]).then_inc(sem, 16)
                s.wait_ge(sem, 16)
    else:
        sem = nc.alloc_semaphore("dma_done")
        nc.sync.dma_start(out=out_ext[:], in_=const[:]).then_inc(sem, 16)

    _programs[key] = (nc, list(range(N_CORES)))
    return _programs[key]


def _run(conservative: bool = False) -> np.ndarray | None:
    """Run the SPMD program on cores 0-7; gather; verify; None on mismatch."""
    from concourse.bass_utils import run_bass_kernel_spmd

    nc, core_ids = _build(conservative)
    in_maps = [{} for _ in core_ids]
    res = run_bass_kernel_spmd(nc, in_maps, core_ids)
    shards = [np.asarray(res.results[i]["out"]).reshape(-1) for i in core_ids]
    full = np.concatenate(shards).reshape(1, C).astype(np.float32, copy=False)
    if np.array_equal(full, np.full((1, C), np.float32(_NEG_LOG_C))):
        return full
    return None


def kernel(**inputs: np.ndarray) -> np.ndarray:
    """Full (unsharded) inputs in, full [1, 32768] float32 output out."""
    # Light shape validation of the full inputs (their values are provably
    # irrelevant to the output — see module docstring).
    x = inputs.get("x")
    if x is not None:
        x = np.asarray(x)
        assert x.shape[1] == C, f"expected C={C} events, got x shape {x.shape}"

    out = _run(conservative=False)
    if out is None:  # torn readback (never observed) — retry conservatively
        out = _run(conservative=True)
    if out is None:
        raise RuntimeError("device output failed verification on both paths")
    return out


# revision 6
# speedup vs baseline: 1.0605x; 1.0381x over previous
"""Trainium2 Bass kernel for nn_ChoiceRNN_79989470921261.

Reference computation (see the problem's reference.py):

    e_user = W_user[idx[:, :, 0]]                      # [1, C, K]
    e_item = W_item[idx[:, :, 1]]                      # [1, C, K]
    interaction = sum(e_user * e_item, axis=2)         # [1, C, 1]
    inp = concat([x, e_user, e_item, interaction])[0]  # [C, 13]
    def step(a0, row):
        h = concat([row, a0])                          # [14] (13 feats + a0)
        h = relu(W1 @ h); h = relu(W2 @ h)
        a0_new = log_softmax(W3 @ h, axis=0)           # [1]
        return a0_new, a0_new[0]
    _, activations = lax.scan(step, zeros(1), inp)     # [C]
    return log_softmax(activations, axis=0)            # [1, C]

Exact algebraic simplification (bit-exact in IEEE float32, not an
approximation):

  1. ``W3 @ h`` has shape [1].  ``log_softmax`` over a length-1 axis is
     ``v - max(v) - log(sum(exp(v - max(v)))) = 0 - log(exp(0)) = 0``
     EXACTLY for any finite v (``v - v`` and ``log(1.0)`` are exact IEEE
     ops).  The reference's own comment notes this:
     "log_softmax of 1 elem == 0".
  2. Therefore the scan carry a0 is exactly 0.0f at every step and every
     emitted activation is exactly 0.0f, independent of x / idx / the
     embedding tables / W1 / W2 / W3.
  3. The final ``log_softmax(zeros(C))`` with C = 32768 = 2**15:
     max = 0, shifted = 0, sum(exp(0)) over C elements = 32768.0 exactly
     (all partial sums of 1.0 are integers < 2**24, exact in fp32 under
     any summation order), so every output element is
     ``-log(32768.0f)`` = -10.397207f (bit pattern 0xC1265B06).

So the network constant-folds: the full-precision output is the constant
``-log(32768)`` broadcast over [1, 32768] for ANY input values (verified
bit-exact against the jax reference).  The device kernel therefore only
has to materialize that constant — the roofline for this problem is the
128 KiB output write.

Device kernel (per core): the 16 KiB constant shard lives in an inline
(NEFF-embedded) DRAM tensor; one hardware-DGE DMA on the sync engine
copies it to the output buffer.  DMA completion is enforced by the
compiler-emitted end-of-program drain on the sync engine, so no explicit
semaphore wait sits on the critical path (measured: ~8.8 us NEFF exec vs
~11.3 us with a Block + explicit wait).  Full attribution of the
measured window (gauge counts first bass instruction -> end of epilogue):
~0.7 us DMA issue (the only kernel-attributable cost), ~1.2 us bass-init
emissions, and ~7 us of walrus-mandated tail that every kernel in this
harness pays — an unconditional per-semaphore reset epilogue over
S[54..255] (~50 EVENT_SEMAPHOREs per engine, ~4.8 us on the slowest
engine) plus two all-engine barriers.  Verified by instruction-level
trace analysis; no vanilla-API lever (Block flags, Bass constructor
options, DMA shapes/AP .opt()) moves it.  As a safety net the host
verifies the gathered output and falls back to a conservative program
(Block + explicit semaphore wait) if it ever saw a torn readback —
never observed in testing.

Sharding: data-parallel over the sequence dimension c (per the hint);
core i produces output positions [i*4096, (i+1)*4096).  The host
concatenates the 8 shards.
"""

import numpy as np

C = 32768
N_CORES = 8
PER_CORE = C // N_CORES            # 4096 output elements per core

# -log(32768.0f) in float32: bit pattern 0xC1265B06, matching the
# reference's log_softmax(zeros(32768)) exactly (see derivation above).
_NEG_LOG_C = float(-np.log(np.float32(C)))

_programs = {}  # name -> (nc, core_ids); cached so repeat calls reuse the BIR


def _build(conservative: bool = False):
    """Build the per-core Bass program once.

    Fast path: single fire-and-forget HWDGE DMA (completion enforced by the
    NEFF epilogue's sync-engine drain).  Conservative path: same DMA inside
    a Block with an explicit semaphore wait before the end barrier.
    """
    key = "conservative" if conservative else "fast"
    if key in _programs:
        return _programs[key]

    import concourse.bass as bass
    import concourse.mybir as mybir

    nc = bass.Bass()
    out_ext = nc.declare_dram_parameter("out", [PER_CORE], mybir.dt.float32,
                                        isOutput=True)
    const = nc.inline_tensor(np.full((PER_CORE,), _NEG_LOG_C, np.float32),
                             name="cval")
    if conservative:
        with (nc.semaphore("dma_done") as sem, nc.Block() as block):
            @block.sync
            def _(s):
                s.dma_start(out=out_ext[:], in_=const[:]).then_inc(sem, 16)
                s.wait_ge(sem, 16)
    else:
        sem = nc.alloc_semaphore("dma_done")
        nc.sync.dma_start(out=out_ext[:], in_=const[:]).then_inc(sem, 16)

    _programs[key] = (nc, list(range(N_CORES)))
    return _programs[key]


def _ensure_axon_profile_hook():
    """bass_utils' axon trace path — also triggered by BASS_TRACE=1 in the
    environment — does ``from antenv.axon_hooks import ...``, which some
    agent images lack; that would crash kernel() with ModuleNotFoundError.
    If the module is missing, supply it: with the real ctypes NTFF hook when
    the axon .so exports the profile symbols (so a trace-requesting harness
    gets real measurements), else with a None hook (bass_utils then logs
    "hook isn't registered" and runs without tracing).  No-op when the
    image already provides antenv.axon_hooks."""
    try:
        import antenv.axon_hooks  # noqa: F401
        return
    except ImportError:
        pass
    try:
        import sys
        import types

        import antenv

        mod = types.ModuleType("antenv.axon_hooks")
        _hook = [None]
        mod.set_axon_ntff_profile_hook = lambda h: _hook.__setitem__(0, h)
        mod.get_axon_ntff_profile_hook = lambda: _hook[0]
        sys.modules["antenv.axon_hooks"] = mod
        antenv.axon_hooks = mod
        try:
            from trn_agent_boot.trn_boot import _ntff_profile_via_ctypes

            hook = _ntff_profile_via_ctypes("/opt/axon/libaxon_pjrt.so")
            if hook is not None:
                mod.set_axon_ntff_profile_hook(hook)
        except Exception:
            pass  # None hook: tracing skipped gracefully, execution works
    except Exception:
        pass  # best-effort shim; never block the actual kernel run


def _run(conservative: bool = False) -> np.ndarray | None:
    """Run the SPMD program on cores 0-7; gather; verify; None on mismatch."""
    _ensure_axon_profile_hook()
    from concourse.bass_utils import run_bass_kernel_spmd

    nc, core_ids = _build(conservative)
    in_maps = [{} for _ in core_ids]
    res = run_bass_kernel_spmd(nc, in_maps, core_ids)
    shards = [np.asarray(res.results[i]["out"]).reshape(-1) for i in core_ids]
    full = np.concatenate(shards).reshape(1, C).astype(np.float32, copy=False)
    if np.array_equal(full, np.full((1, C), np.float32(_NEG_LOG_C))):
        return full
    return None


def kernel(**inputs: np.ndarray) -> np.ndarray:
    """Full (unsharded) inputs in, full [1, 32768] float32 output out."""
    # Light shape validation of the full inputs (their values are provably
    # irrelevant to the output — see module docstring).
    x = inputs.get("x")
    if x is not None:
        x = np.asarray(x)
        assert x.shape[1] == C, f"expected C={C} events, got x shape {x.shape}"

    out = _run(conservative=False)
    if out is None:  # torn readback (never observed) — retry conservatively
        out = _run(conservative=True)
    if out is None:
        raise RuntimeError("device output failed verification on both paths")
    return out
